# revision 5
# baseline (speedup 1.0000x reference)
"""GCN layer v3 on 8 Trainium2 NeuronCores.

Changes vs v2:
- feat pre-scaled by norm_src on host (fp16 rows); norm_dst applied by the
  Act engine during the PSUM->SBUF output copies (per-partition scale).
- One-hot S built by ONE broadcast tensor_tensor (is_equal) per piece of
  <=24 entries: in0 = dl tile broadcast along iota axis (stride-0 inner),
  in1 = iota with stride-0 entry axis. Single-ALU-op, fp16.
- Variable entry windows: each 128-edge chunk gets ceil(span/128) entries
  at unaligned 128-wide windows starting at the chunk's min dloc
  (clamped to 384); edges are uniquely assigned to entries (others 999).
- psT PSUM bank zero-initialised by Act copy of a zeros tile; all edge
  matmuls accumulate with start=False.
- Gather calls merged: one call per (sb512, range) group (MAX_CALL from
  ring probe); idx loads merged to one DMA per super-block.
- Bias added only if nonzero (Pool tensor_tensor add).
"""
import numpy as np

N_NODES = 100000
N_EDGES = 1600000
F = 128
N_CORES = 8
OWN = 12544
SB = 1024
NSB = 13
RANGE = 32768
NRANGES = 4
MAX_CALL = 896      # dma_gather single-call cap (ring = 64 descs incl sem)
PIECE = 24          # one-hot entries per DVE instruction


def _install_walrus_passes():
    import concourse.bass_utils as bu

    def patched(tmpdir, inp="bir.json", outp="file.neff", arch=None, *, dve_root=None):
        from pathlib import Path
        cmd = [
            bu.get_walrus_driver(),
            "--pass",
            "birverifier,dynamic_dma_scan,runtime_memory_reservation,"
            "dynamic_dma_setup,lower_act,lower_dve,lower_ap_offset,"
            "codegen,neff_packager",
            "-i", inp,
            "--neff-output-filename", outp,
            "--enable-birsim=true",
            "--mem-mode=physical",
            "--policy=0",
            "--enable-ldw-opt=false",
            "--assign-static-dmas-to-sp=false",
            "--dram-page-size=256",
            "--enable-neff-debug-info=true",
            "--jobs", "8",
            "--dynamic-dma-scratch-size-per-partition=16384",
            *bu.get_walrus_args(
                bu.get_bir_arch(tmpdir, inp) if arch is None else arch,
                tmpdir, dve_root=dve_root,
            ),
        ]
        result = bu.run_command(cmd, cwd=tmpdir)
        if result is not None:
            (Path(tmpdir) / "log.txt").write_text(result.stdout)
        return f"{tmpdir}/{outp}"

    bu.bir_verify_and_optimise = patched


def _pack_idx_wrap(idx_i16: np.ndarray, cap: int) -> np.ndarray:
    w = np.zeros((16, cap // 16), np.int16)
    j = np.arange(len(idx_i16))
    w[j % 16, j // 16] = idx_i16
    return np.tile(w, (8, 1))


def _preprocess(src: np.ndarray, dst: np.ndarray):
    """Returns (prog, per_core, norm_src, norm_dst_percore)."""
    src = np.asarray(src).astype(np.int64)
    dst = np.asarray(dst).astype(np.int64)

    out_deg = np.bincount(src, minlength=N_NODES).astype(np.float32)
    in_deg = np.bincount(dst, minlength=N_NODES).astype(np.float32)
    norm_src = 1.0 / np.sqrt(np.clip(out_deg, 1.0, None))
    norm_dst = 1.0 / np.sqrt(np.clip(in_deg, 1.0, None))

    core = np.minimum(dst // OWN, N_CORES - 1)
    dst_local = dst - core * OWN
    sb = dst_local // SB
    dloc = dst_local - sb * SB
    rng = src // RANGE

    sizes = np.zeros((N_CORES, NSB, NRANGES), np.int64)
    np.add.at(sizes, (core, sb, rng), 1)
    gmax = sizes.max(axis=0)
    gpad = ((gmax + 127) // 128) * 128

    plan = []               # (sb, r, chunk_offset, n_idx) gather calls
    total_chunks = 0
    group_chunk0 = {}
    for s in range(NSB):
        for r in range(NRANGES):
            n = int(gpad[s, r])
            group_chunk0[(s, r)] = total_chunks
            if n == 0:
                continue
            off = 0
            while off < n:
                take = min(MAX_CALL, n - off)
                plan.append((s, r, total_chunks + off // 128, take))
                off += take
            total_chunks += n // 128

    chunks_per_sb = np.array(
        [sum(int(gpad[s, r]) // 128 for r in range(NRANGES)) for s in range(NSB)],
        np.int64)
    idx_cols = total_chunks * 8

    # per-core padded streams
    dlmin = np.full(total_chunks, SB, np.int64)
    dlmax = np.full(total_chunks, -1, np.int64)
    core_streams = []
    for k in range(N_CORES):
        m = core == k
        e_sb, e_rng = sb[m], rng[m]
        e_src, e_dl = src[m], dloc[m]
        order = np.lexsort((e_dl, e_rng, e_sb))
        e_sb, e_rng = e_sb[order], e_rng[order]
        e_src, e_dl = e_src[order], e_dl[order]

        idx_stream = np.zeros(total_chunks * 128, np.int16)
        dl_stream = np.full(total_chunks * 128, -1, np.int64)  # -1 = pad

        gsizes = np.zeros((NSB, NRANGES), np.int64)
        np.add.at(gsizes, (e_sb, e_rng), 1)
        acc = 0
        for s in range(NSB):
            for r in range(NRANGES):
                n_real = int(gsizes[s, r])
                if n_real == 0:
                    continue
                p0 = group_chunk0[(s, r)] * 128
                sl = slice(p0, p0 + n_real)
                idx_stream[sl] = (e_src[acc:acc + n_real] - r * RANGE).astype(np.int16)
                dl_stream[sl] = e_dl[acc:acc + n_real]
                acc += n_real
        core_streams.append((idx_stream, dl_stream))

        dl2 = dl_stream.reshape(total_chunks, 128)
        v2 = dl2 >= 0
        has = v2.any(axis=1)
        cmin = np.where(has, np.where(v2, dl2, SB).min(axis=1), SB)
        cmax = np.where(has, np.where(v2, dl2, -1).max(axis=1), -1)
        dlmin = np.minimum(dlmin, cmin)
        dlmax = np.maximum(dlmax, cmax)

    # entries: per chunk, 128-wide windows covering [dlmin, dlmax]; a
    # window must not cross a 512-col PSUM bank boundary, so bases are
    # clamped to 384 within each bank and bank lines force new entries.
    def chunk_bases(lo, hi):
        bases = []
        b = int(lo)
        b = min(b, (b // 512) * 512 + 384)
        while True:
            bases.append(b)
            top = b + 128
            if top > hi:
                break
            nb = top
            nb = min(nb, (nb // 512) * 512 + 384)
            if nb <= b:
                nb = b + 128
            b = nb
        return bases

    has_any = dlmax >= 0
    entry_chunk = []
    entry_base = []
    entry_off = np.zeros(total_chunks + 1, np.int64)
    for c in range(total_chunks):
        entry_off[c] = len(entry_chunk)
        if has_any[c]:
            bs = chunk_bases(dlmin[c], dlmax[c])
        else:
            bs = [0]
        for b in bs:
            entry_chunk.append(c)
            entry_base.append(b)
        entry_off[total_chunks] = len(entry_chunk)
    entry_chunk = np.array(entry_chunk, np.int64)
    entry_base = np.array(entry_base, np.int64)
    total_entries = len(entry_chunk)

    # per-sb entry ranges
    sb_chunk0 = np.concatenate([[0], np.cumsum(chunks_per_sb)])
    sb_entry0 = entry_off[sb_chunk0]

    # per-core dl buffers [128, total_entries] fp16 (999 = no match)
    per_core = []
    for k in range(N_CORES):
        idx_stream, dl_stream = core_streams[k]
        pos = np.arange(total_chunks * 128)
        c_of = pos // 128
        p_of = pos % 128
        valid = dl_stream >= 0
        dl_v = dl_stream[valid]
        c_v = c_of[valid]
        p_v = p_of[valid]
        # assign each edge to the first of its chunk's windows containing it
        e_v = entry_off[c_v].copy()
        rel = dl_v - entry_base[e_v]
        for _ in range(8):
            over = rel >= 128
            if not over.any():
                break
            e_v[over] += 1
            rel = dl_v - entry_base[e_v]
        assert rel.min() >= 0 and rel.max() < 128
        dl_buf = np.full((total_entries, 128), 999.0, np.float16)
        dl_buf[e_v, p_v] = rel.astype(np.float16)

        idx_buf = np.zeros((128, idx_cols), np.int16)
        for s, r, c0, n in plan:
            seg = idx_stream[c0 * 128: c0 * 128 + n]
            idx_buf[:, c0 * 8: c0 * 8 + n // 16] = _pack_idx_wrap(seg, n)
        per_core.append((idx_buf, np.ascontiguousarray(dl_buf.T)))

    # per-core norm_dst table [128, NSB*4]: col s*4+j, partition p ->
    # norm_dst of global node k*OWN + s*512 + j*128 + p
    nd_tab = np.zeros((N_CORES, 128, NSB * (SB // 128)), np.float32)
    nd_pad = np.concatenate([norm_dst, np.zeros(N_CORES * OWN + SB, np.float32)])
    for k in range(N_CORES):
        base = k * OWN
        idx = base + np.arange(NSB * SB)
        vals = nd_pad[idx].reshape(NSB * (SB // 128), 128)
        nd_tab[k] = vals.T

    prog = {
        "plan": plan,
        "chunks_per_sb": chunks_per_sb,
        "total_chunks": total_chunks,
        "idx_cols": idx_cols,
        "entry_chunk": entry_chunk,
        "entry_base": entry_base,
        "sb_chunk0": sb_chunk0,
        "sb_entry0": sb_entry0,
        "total_entries": total_entries,
    }
    return prog, per_core, norm_src, nd_tab


def _build_program(prog, with_bias):
    import concourse.bacc as bacc
    import concourse.mybir as mybir
    import concourse.tile as tile
    from concourse.ap import AP

    plan = prog["plan"]
    chunks_per_sb = prog["chunks_per_sb"]
    total_chunks = prog["total_chunks"]
    idx_cols = prog["idx_cols"]
    entry_chunk = prog["entry_chunk"]
    entry_base = prog["entry_base"]
    sb_chunk0 = prog["sb_chunk0"]
    sb_entry0 = prog["sb_entry0"]
    total_entries = prog["total_entries"]

    nc = bacc.Bacc(num_swdge_queues=4)
    feat_d = nc.declare_dram_parameter("feat16", [N_NODES, F], mybir.dt.float16, isOutput=False)
    w_d = nc.declare_dram_parameter("w16", [F, F], mybir.dt.float16, isOutput=False)
    bias_d = nc.declare_dram_parameter("biasb", [128, SB], mybir.dt.float32, isOutput=False)
    iota_d = nc.declare_dram_parameter("iota", [128, 128], mybir.dt.float16, isOutput=False)
    idx_d = nc.declare_dram_parameter("idxb", [128, idx_cols], mybir.dt.int16, isOutput=False)
    dl_d = nc.declare_dram_parameter("dlb", [128, total_entries], mybir.dt.float16, isOutput=False)
    nd_d = nc.declare_dram_parameter("ndst", [128, NSB * (SB // 128)], mybir.dt.float32, isOutput=False)
    out_d = nc.declare_dram_parameter("out", [NSB * SB, F], mybir.dt.float32, isOutput=True)

    ranges = [(r * RANGE, min((r + 1) * RANGE, N_NODES)) for r in range(NRANGES)]

    with tile.TileContext(nc) as tc:
        with (
            tc.tile_pool(name="const", bufs=1) as constp,
            tc.tile_pool(name="et", bufs=2) as etp,
            tc.tile_pool(name="ix", bufs=3) as ixp,
            tc.tile_pool(name="dl", bufs=3) as dlp,
            tc.tile_pool(name="s", bufs=4) as sp,
            tc.tile_pool(name="aggs", bufs=2) as aggsp,
            tc.tile_pool(name="outs", bufs=2) as outsp,
            tc.tile_pool(name="ps", bufs=2, space="PSUM") as psp,
            tc.tile_pool(name="ps2", bufs=2, space="PSUM") as ps2p,
        ):
            w_t = constp.tile([F, F], mybir.dt.float16)
            nc.sync.dma_start(w_t[:], w_d[:])
            bias_t = constp.tile([128, SB], mybir.dt.float32)
            nc.sync.dma_start(bias_t[:], bias_d[:])
            iota_t = constp.tile([128, 128], mybir.dt.float16)
            nc.sync.dma_start(iota_t[:], iota_d[:])
            nd_t = constp.tile([128, NSB * (SB // 128)], mybir.dt.float32)
            nc.sync.dma_start(nd_t[:], nd_d[:])
            zeros_t = constp.tile([128, SB], mybir.dt.float32)
            nc.vector.memset(zeros_t[:], 0.0)

            sb_plan = {}
            for s, r, c0, n in plan:
                sb_plan.setdefault(s, []).append((r, c0, n))

            call_counter = [0]
            for s in range(NSB):
                nch = int(chunks_per_sb[s])
                if nch == 0:
                    continue
                cb = int(sb_chunk0[s])
                e0 = int(sb_entry0[s])
                e1 = int(sb_entry0[s + 1])
                ne = e1 - e0

                # gather the super-block's edge rows (one call per range)
                et = etp.tile([128, nch * F], mybir.dt.float16)
                ix = ixp.tile([128, nch * 8], mybir.dt.int16)
                nc.sync.dma_start(ix[:], idx_d[:, cb * 8: (cb + nch) * 8])
                for r, c0, n in sb_plan[s]:
                    lo, hi = ranges[r]
                    rel = c0 - cb
                    nc.gpsimd.dma_gather(
                        out_ap=et[:, rel * F: (rel + n // 128) * F].rearrange(
                            "p (c e) -> p c e", e=F),
                        in_ap=feat_d[lo:hi, :],
                        idxs_ap=ix[:, rel * 8: rel * 8 + n // 16],
                        num_idxs=n,
                        num_idxs_reg=n,
                        elem_size=F,
                        queue_num=call_counter[0] % 4,
                    )
                    call_counter[0] += 1

                # dl metadata for this super-block's entries
                dl = dlp.tile([128, ne], mybir.dt.float16)
                nc.sync.dma_start(dl[:], dl_d[:, e0:e1])

                # zero-init PSUM bank, then accumulate entry matmuls
                psT = psp.tile([128, SB], mybir.dt.float32, space="PSUM")
                nc.scalar.copy(psT[:], zeros_t[:])
                for p0 in range(0, ne, PIECE):
                    pe = min(p0 + PIECE, ne)
                    np_ = pe - p0
                    st = sp.tile([128, PIECE * 128], mybir.dt.float16, tag="s")
                    b = dl[:, p0:pe].to_broadcast([128, np_, 128])
                    a = iota_t[:]
                    i2 = AP(a.tensor, a.offset,
                            [list(a.ap[0]), [0, np_], list(a.ap[1])])
                    nc.vector.tensor_tensor(
                        out=st[:, : np_ * 128].rearrange("p (c j) -> p c j", j=128),
                        in0=b, in1=i2, op=mybir.AluOpType.is_equal,
                    )
                    for k in range(p0, pe):
                        c = int(entry_chunk[e0 + k]) - cb
                        base = int(entry_base[e0 + k])
                        nc.tensor.matmul(
                            out=psT[:, base: base + 128],
                            lhsT=et[:, c * F: (c + 1) * F],
                            rhs=st[:, (k - p0) * 128: (k - p0 + 1) * 128],
                            start=False,
                            stop=(k == ne - 1),
                        )

                aggT = aggsp.tile([128, SB], mybir.dt.float16)
                nc.scalar.copy(aggT[:], psT[:])
                ps2 = ps2p.tile([128, SB], mybir.dt.float32, space="PSUM")
                for j in range(SB // F):
                    nc.tensor.matmul(
                        out=ps2[:, j * F: (j + 1) * F],
                        lhsT=aggT[:, j * F: (j + 1) * F],
                        rhs=w_t[:],
                        start=True,
                        stop=True,
                    )
                # PSUM -> SBUF with per-partition norm_dst scale
                ot = outsp.tile([128, SB], mybir.dt.float32)
                for j in range(SB // F):
                    nc.scalar.activation(
                        ot[:, j * F: (j + 1) * F],
                        ps2[:, j * F: (j + 1) * F],
                        mybir.ActivationFunctionType.Copy,
                        scale=nd_t[:, s * (SB // F) + j: s * (SB // F) + j + 1],
                    )
                if with_bias:
                    nc.gpsimd.tensor_tensor(
                        out=ot[:], in0=ot[:], in1=bias_t[:],
                        op=mybir.AluOpType.add,
                    )
                nc.sync.dma_start(
                    out_d[s * SB: (s + 1) * SB, :].rearrange("(j p) f -> p j f", p=128),
                    ot[:].rearrange("p (j f) -> p j f", f=F),
                )
    nc.finalize()
    return nc


def kernel(feat, weight, bias, src, dst):
    _install_walrus_passes()
    from concourse.bass_utils import run_bass_kernel_spmd

    feat = np.asarray(feat, dtype=np.float32)
    weight = np.asarray(weight, dtype=np.float32)
    bias = np.asarray(bias, dtype=np.float32)

    prog, per_core, norm_src, nd_tab = _preprocess(src, dst)
    feat16 = np.ascontiguousarray((feat * norm_src[:, None]).astype(np.float16))
    w16 = np.ascontiguousarray(weight.astype(np.float16))
    with_bias = bool(np.any(bias != 0.0))
    nc = _build_program(prog, with_bias)

    bias_b = np.broadcast_to(np.tile(bias, SB // F)[None, :], (128, SB)).copy()
    iota = np.broadcast_to(np.arange(128, dtype=np.float16)[None, :], (128, 128)).copy()

    in_maps = []
    for k in range(N_CORES):
        idx_buf, dl_buf = per_core[k]
        in_maps.append({
            "feat16": feat16,
            "w16": w16,
            "biasb": bias_b,
            "iota": iota,
            "idxb": idx_buf,
            "dlb": dl_buf,
            "ndst": np.ascontiguousarray(nd_tab[k]),
        })
    res = run_bass_kernel_spmd(nc, in_maps, list(range(N_CORES)))
    out = np.empty((N_CORES * OWN, F), np.float32)
    for k in range(N_CORES):
        out[k * OWN: (k + 1) * OWN] = res.results[k]["out"][:OWN]
    return out[:N_NODES]
